# revision 2
# baseline (speedup 1.0000x reference)
"""Trainium2 Bass kernel for nn_DataReuploadingTorso (8-qubit data-reuploading
circuit, batch 16384), fp16 pipeline v5.

Math: each PennyLane Rot = RZ H RZ H RZ; the circuit becomes 17 diagonal
phase steps interleaved with H^{x8}. Per step k: st' = D_k (*) (H st).

Device mapping (per core, batch 2048; state [128 part = low 7 bits,
(h, batch) free]):
- H-transform: butterfly folded into PSUM accumulation - per 512-chunk
  t_u = M a + M b, t_v = M a + (-M) b with M = H^{x7}/16 fp16 stationary
  ([M | -M] table), fp16 state moving (1 cyc/row).
- PSUM -> SBUF fp16 casts on the ACT engine (Copy), so every DVE
  TensorTensor op is all-16-bit packed -> 2x_1p perf mode.
- Phase pipeline (2-stage lookahead, fully off the state critical path):
  Phi (turns) via fp32r matmul; round n = rne(Phi) as an ACT Copy to
  int16 (hw round-to-nearest); f = Phi - n via one DVE STT; S = Sin(2pi f)
  and C = sin(pi/2 - 2pi |f|) (ACT Abs + Sin with scale=-2pi, bias=pi/2)
  - no second range reduction for cos. Batch-indep steps broadcast host
  cos/sin (sin halves DVE-copy at 4x, cos halves ACT-copy).
- Multiply: pA = t_re (*) [S|C], pB = t_im (*) [S|C] (fp16 2x), then
  per-half sub/add -> new state (fp16).
- Emission order per stage: state MMs + casts lead every queue; next
  stage's Sin/Abs/Sin reads an fsc finished one stage earlier; rounds
  for stage k+2 trail. SC bufs=3 / fsc bufs=2 carry the lookahead.

Sharding: pure data-parallel over batch across 8 NeuronCores (2048 each).
"""
import os

import numpy as np

import concourse.bass as bass
import concourse.mybir as mybir
import concourse.tile as tile
from concourse.bass_utils import run_bass_kernel_spmd

N_CORES = 8
B_TOTAL = 16384
B_CORE = B_TOTAL // N_CORES      # 2048
H = B_CORE                       # one half (h) slab of the free dim
W2 = 2 * B_CORE                  # 4096 free cols: [h=0 batch | h=1 batch]
NSTEP = 17
DIM = 256
N_Q = 8

F32 = mybir.dt.float32
F32R = mybir.dt.float32r
F16 = mybir.dt.float16
I16 = mybir.dt.int16
AOT = mybir.AluOpType
ACTF = mybir.ActivationFunctionType

PI = float(np.pi)
TWO_PI = float(np.float32(2.0 * np.pi))
HALF_PI = float(np.pi / 2.0)

KREP = int(os.environ.get("KREP", "1"))
ROUND_MODE = os.environ.get("ROUND_MODE", "act")      # act | dve | split
CAST_IM = os.environ.get("CAST_IM", "act")            # act | skip | pool
PHASE_LATE = os.environ.get("PHASE_LATE", "0") == "1"
COS_VIA = os.environ.get("COS_VIA", "act")            # act (Abs+Sin) | dve (TS)
POOL_BFLY = int(os.environ.get("POOL_BFLY", "0"))     # 0/2/4 bfly units on Pool

BI_STEPS = (2, 6, 10, 14)
BD_STEPS = tuple(k for k in range(NSTEP) if k not in BI_STEPS)  # 13 steps
BD_POS = {k: i for i, k in enumerate(BD_STEPS)}


# ----------------------------------------------------------------- host tables
def _build_host_tables(theta, omega):
    """W (13, 13*256) phase weights (turns) for the 13 batch-dep steps;
    CS (128, 16) cos/sin for the 4 batch-indep steps; M16 (128,128) fp16 =
    H^{x7}/16; Z16 (128, 16) fp16 = [Z_h0 | Z_h1] PauliZ stationaries."""
    theta = np.asarray(theta, np.float64)              # (8, 5, 3)
    omega = np.asarray(omega, np.float64).reshape(5, 8, 3)

    idx = np.arange(DIM)
    beta = np.stack([(idx >> (7 - q)) & 1 for q in range(N_Q)], 0)   # (8, 256)
    sgn = (2 * beta - 1).astype(np.float64)

    def czterm(pairs):
        t = np.zeros(DIM)
        for a, b in pairs:
            t += np.pi * (beta[a] * beta[b])
        return t
    cz_even = czterm([(0, 1), (2, 3), (4, 5), (6, 7)])
    cz_odd = czterm([(1, 2), (3, 4), (5, 6)])

    steps = []
    for l in range(4):
        A = np.zeros((12, 8)); k = np.zeros(8)
        for q in range(8):
            A[3 * (q % 4) + 1, q] = omega[l, q, 1]
        steps.append((A, k, None))
        A = np.zeros((12, 8)); k = np.zeros(8)
        for q in range(8):
            A[3 * (q % 4) + 2, q] = omega[l, q, 2]
            k[q] = theta[q, l, 0]
        steps.append((A, k, None))
        A = np.zeros((12, 8)); k = theta[:, l, 1].copy()
        steps.append((A, k, None))                     # batch-independent
        A = np.zeros((12, 8)); k = theta[:, l, 2].copy()
        for q in range(8):
            A[3 * (q % 4) + 0, q] = omega[l + 1, q, 0]
        if l + 1 == 4:
            k = k + theta[:, 4, 0]
        steps.append((A, k, cz_even if l % 2 == 0 else cz_odd))
    A = np.zeros((12, 8)); k = theta[:, 4, 1].copy()
    for q in range(8):
        A[3 * (q % 4) + 1, q] = omega[4, q, 1]
    steps.append((A, k, None))

    inv2pi = 1.0 / (2.0 * np.pi)
    W = np.zeros((13, len(BD_STEPS) * DIM))
    CS = np.zeros((128, 4 * len(BI_STEPS)), np.float32)
    for i, (A, k, cz) in enumerate(steps):
        c = k @ (sgn * 0.5)
        if cz is not None:
            c = c + cz
        if i in BI_STEPS:
            j = BI_STEPS.index(i)
            for h in (0, 1):
                CS[:, 4 * j + 0 + h] = np.cos(c[h * 128:(h + 1) * 128])
                CS[:, 4 * j + 2 + h] = np.sin(c[h * 128:(h + 1) * 128])
        else:
            base = BD_POS[i] * DIM
            W[:12, base:base + DIM] = (A @ (sgn * 0.5)) * inv2pi
            W[12, base:base + DIM] = \
                (np.mod(c + np.pi, 2 * np.pi) - np.pi) * inv2pi

    sp = np.arange(128)
    pop = np.zeros((128, 128), np.int64)
    for q in range(7):
        pop += np.outer((sp >> q) & 1, (sp >> q) & 1)
    M1 = (np.where(pop % 2 == 0, 1.0, -1.0) / 16.0)
    M16 = np.concatenate([M1, -M1], 1).astype(np.float16)   # [M | -M]

    Z = np.stack([1.0 - 2.0 * ((idx >> (7 - q)) & 1) for q in range(8)], 1)
    Z16 = np.concatenate([Z[:128], Z[128:]], 1).astype(np.float16)  # (128,16)
    return W.astype(np.float32), CS.astype(np.float16), M16, Z16


# -------------------------------------------------------------- device program
def _legalize_waits(nc, limit=1, limit_other=None):
    """walrus codegen allows only one embedded sync-wait on several TRN2
    instruction encodings. Hoist excess waits onto same-engine NoOps."""
    if limit_other is None:
        limit_other = limit
    one_wait = (mybir.InstMatmult, mybir.InstTensorScalarPtr)

    def fix_block(blk):
        new_insts = []
        for ins in blk.instructions:
            lim = limit if isinstance(ins, one_wait) else limit_other
            si = getattr(ins, "sync_info", None)
            waits = list(si.on_wait) if si and si.on_wait else []
            if len(waits) > lim:
                keep = waits[-lim:]
                for j, w in enumerate(waits[:-lim]):
                    new_insts.append(mybir.InstNoOp(
                        name=f"{ins.name}-w{j}",
                        engine=ins.engine,
                        sync_info=mybir.SyncInfo(on_wait=[w], on_update=[]),
                    ))
                si.on_wait = keep
            new_insts.append(ins)
        blk.instructions = new_insts
        for sb in getattr(blk, "blocks", None) or []:
            fix_block(sb)
    for f in nc.m.functions:
        for blk in f.blocks:
            fix_block(blk)


def _build_program():
    nc = bass.Bass("TRN2", target_bir_lowering=False, debug=False,
                   enable_asserts=False, num_devices=N_CORES)

    # const APs for ACT biases (bias floats must be pre-registered)
    for val in (HALF_PI,):
        t = nc.alloc_sbuf_tensor(f"const-{val:.6f}", [128, 1], F32)
        nc.gpsimd.memset(t.ap(), val)
        nc.const_aps.aps[(F32, val)] = t.ap()

    xT_d = nc.dram_tensor("xT", [13, B_CORE], F32, kind="ExternalInput")
    W_d = nc.dram_tensor("W", [13, len(BD_STEPS) * DIM], F32,
                         kind="ExternalInput")
    M_d = nc.dram_tensor("M", [128, 256], F16, kind="ExternalInput")
    Z_d = nc.dram_tensor("Zt", [128, 16], F16, kind="ExternalInput")
    CS_d = nc.dram_tensor("CS", [128, 4 * len(BI_STEPS)], F16,
                          kind="ExternalInput")
    out_d = nc.dram_tensor("out", [8, B_CORE], F32, kind="ExternalOutput")

    with tile.TileContext(nc) as tc:
        with (
            tc.tile_pool(name="consts", bufs=1) as consts,
            tc.tile_pool(name="psum", bufs=2, space="PSUM") as pp,
            tc.tile_pool(name="st", bufs=2) as st_pool,
            tc.tile_pool(name="t16", bufs=1) as t_pool,
            tc.tile_pool(name="sc", bufs=3) as sc_pool,
            tc.tile_pool(name="fsc", bufs=2) as fsc_pool,
            tc.tile_pool(name="fab", bufs=1) as fab_pool,
            tc.tile_pool(name="iq", bufs=2) as iq_pool,
            tc.tile_pool(name="prod", bufs=1) as prod_pool,
        ):
            xT = consts.tile([13, B_CORE], F32R, tag="xT")
            nc.sync.dma_start(xT[:], xT_d[:].bitcast(F32R))
            Wt = consts.tile([13, len(BD_STEPS) * DIM], F32R, tag="W")
            nc.sync.dma_start(Wt[:], W_d[:].bitcast(F32R))
            Mt = consts.tile([128, 256], F16, tag="M")
            nc.sync.dma_start(Mt[:], M_d[:])
            Zt = consts.tile([128, 16], F16, tag="Z")
            nc.sync.dma_start(Zt[:], Z_d[:])
            cst = consts.tile([128, 4 * len(BI_STEPS)], F16, tag="cs")
            nc.sync.dma_start(cst[:], CS_d[:])

            def phase_tiles(k):
                """Phase matmuls for BD step k: two [128, H] psum tiles
                (turns), one per h."""
                base = BD_POS[k] * DIM
                tiles = []
                for h in (0, 1):
                    qp = pp.tile([128, H], F32, tag="ps", name=f"q{k}h{h}")
                    for c in range(H // 512):
                        nc.tensor.matmul(
                            qp[:, c * 512:(c + 1) * 512],
                            Wt[:, base + h * 128: base + (h + 1) * 128],
                            xT[:, c * 512:(c + 1) * 512],
                            start=True, stop=True)
                    tiles.append(qp)
                return tiles

            def rounds(k, qtiles, fsc):
                """fsc[:, h*H:(h+1)*H] = Phi_h - rne(Phi_h)  (turns)."""
                if ROUND_MODE == "split":
                    qi1 = iq_pool.tile([128, H], I16, tag="iq",
                                       name=f"qi{k}h1")
                    nc.vector.tensor_scalar_add(qi1[:], qtiles[1][:], 0.0)
                    qi0 = iq_pool.tile([128, H], I16, tag="iq",
                                       name=f"qi{k}h0")
                    nc.scalar.activation(qi0[:], qtiles[0][:], ACTF.Copy)
                    nc.vector.scalar_tensor_tensor(
                        fsc[:, H:W2], qtiles[1][:], 0.0,
                        qi1[:], AOT.add, AOT.subtract)
                    nc.vector.scalar_tensor_tensor(
                        fsc[:, 0:H], qtiles[0][:], 0.0,
                        qi0[:], AOT.add, AOT.subtract)
                    return
                for h in (0, 1):
                    qi = iq_pool.tile([128, H], I16, tag="iq",
                                      name=f"qi{k}h{h}")
                    if ROUND_MODE == "act":
                        nc.scalar.activation(qi[:], qtiles[h][:], ACTF.Copy)
                    else:
                        nc.vector.tensor_scalar_add(qi[:], qtiles[h][:], 0.0)
                    nc.vector.scalar_tensor_tensor(
                        fsc[:, h * H:(h + 1) * H], qtiles[h][:], 0.0,
                        qi[:], AOT.add, AOT.subtract)

            def sincos(k, fsc, SC):
                """SC[:,0]=sin(2pi f), SC[:,1]=cos fp16; cos via
                sin(pi/2 - 2pi |f|). Per-half so Abs h0 starts right
                after STT h0."""
                for h in (0, 1):
                    hs = slice(h * H, (h + 1) * H)
                    nc.scalar.activation(SC[:, 0, hs], fsc[:, hs], ACTF.Sin,
                                         scale=TWO_PI)
                    fab = fab_pool.tile([128, H], F32, tag="fab",
                                        name=f"fab{k}h{h}")
                    nc.scalar.activation(fab[:], fsc[:, hs], ACTF.Abs)
                    nc.scalar.activation(SC[:, 1, hs], fab[:], ACTF.Sin,
                                         scale=-TWO_PI, bias=HALF_PI)

            def bi_sc(k, SC):
                """Broadcast host cos/sin columns for batch-indep step k
                (sin halves on DVE, cos halves on ACT to split the load)."""
                j = BI_STEPS.index(k)
                for h in (0, 1):
                    nc.vector.tensor_copy(
                        SC[:, 0, h * H:(h + 1) * H],
                        cst[:, 4 * j + 2 + h:4 * j + 3 + h]
                        .to_broadcast((128, H)))
                    nc.scalar.activation(
                        SC[:, 1, h * H:(h + 1) * H],
                        cst[:, 4 * j + h:4 * j + h + 1]
                        .to_broadcast((128, H)), ACTF.Copy)

            def mm_cast(k, s_re, s_im):
                """Butterfly folded into PSUM accumulation: per chunk
                t_u = M a + M b (start/stop pair), t_v = M a + (-M) b.
                Reads the state tiles directly (no DVE butterfly). The re
                component is ACT-cast to fp16; the im component is cast,
                pool-cast, or left in PSUM for 1x muls per CAST_IM."""
                tr = t_pool.tile([128, W2], F16, tag="tr", name=f"tr{k}")
                ti = t_pool.tile([128, W2], F16, tag="ti", name=f"ti{k}")
                im_ps = []
                for uv in (0, 1):
                    bsl = slice(128, 256) if uv else slice(0, 128)
                    for comp, s_t, t16 in (("r", s_re, tr), ("i", s_im, ti)):
                        ps = pp.tile([128, H], F32, tag="ps",
                                     name=f"s{k}{comp}{uv}")
                        for c in range(H // 512):
                            sl = slice(c * 512, (c + 1) * 512)
                            nc.tensor.matmul(
                                ps[:, sl], Mt[:, 0:128],
                                s_t[:, c * 512:(c + 1) * 512],
                                start=True, stop=False)
                            nc.tensor.matmul(
                                ps[:, sl], Mt[:, bsl],
                                s_t[:, H + c * 512:H + (c + 1) * 512],
                                start=False, stop=True)
                        if comp == "i" and CAST_IM == "skip":
                            im_ps.append(ps)
                        elif comp == "i" and CAST_IM == "pool":
                            nc.gpsimd.tensor_copy(
                                ti[:, uv * H:(uv + 1) * H], ps[:])
                        else:
                            nc.scalar.activation(
                                t16[:, uv * H:(uv + 1) * H], ps[:],
                                ACTF.Copy)
                return tr, (im_ps if CAST_IM == "skip" else ti)

            def dmult(k, t_re, t_im, SC):
                """st' = (C + iS) (*) t, all fp16 2x. Per-half ops so the
                h=0 chain (mul+addsub) runs while h=1 is still in matmul/
                cast. SC/pA/pB are [128, 2, W2]: plane 0 = S, plane 1 = C."""
                pA = prod_pool.tile([128, 2, W2], F16, tag="pA",
                                    name=f"pA{k}")
                pB = prod_pool.tile([128, 2, W2], F16, tag="pB",
                                    name=f"pB{k}")
                n_re = st_pool.tile([128, W2], F16, tag="sre",
                                    name=f"sre{k}")
                n_im = st_pool.tile([128, W2], F16, tag="sim",
                                    name=f"sim{k}")
                for h in (0, 1):
                    hs = slice(h * H, (h + 1) * H)
                    nc.vector.tensor_mul(
                        pA[:, :, hs],
                        t_re[:, hs].unsqueeze(1).to_broadcast((128, 2, H)),
                        SC[:, :, hs])
                    tiv = (t_im[h][:] if isinstance(t_im, list)
                           else t_im[:, hs])
                    nc.vector.tensor_mul(
                        pB[:, :, hs],
                        tiv.unsqueeze(1).to_broadcast((128, 2, H)),
                        SC[:, :, hs])
                    nc.vector.tensor_sub(n_re[:, hs], pA[:, 1, hs],
                                         pB[:, 0, hs])
                    nc.vector.tensor_add(n_im[:, hs], pA[:, 0, hs],
                                         pB[:, 1, hs])
                return n_re, n_im

            for _rep in range(KREP):
                # ---- init: st = e^{i Phi_0} (x16 folded into M + Square)
                q0 = phase_tiles(0)
                fsc0 = fsc_pool.tile([128, W2], F32, tag="fsc", name="fsc0")
                rounds(0, q0, fsc0)
                st_im = st_pool.tile([128, W2], F16, tag="sim", name="sim0")
                nc.scalar.activation(st_im[:], fsc0[:], ACTF.Sin,
                                     scale=TWO_PI)
                st_re = st_pool.tile([128, W2], F16, tag="sre", name="sre0")
                fab0 = fab_pool.tile([128, W2], F32, tag="fab", name="fab0")
                nc.scalar.activation(fab0[:], fsc0[:], ACTF.Abs)
                nc.scalar.activation(st_re[:], fab0[:], ACTF.Sin,
                                     scale=-TWO_PI, bias=HALF_PI)

                # 2-stage phase lookahead: during stage k we emit
                # phase-matmuls+rounds for k+2 and the Sin/Abs/Sin for k+1
                # (whose fsc was finished last stage) - so ACT work never
                # waits on this stage's DVE.
                fsc_by = {}

                def phase_rounds(kk):
                    if kk >= NSTEP or kk in BI_STEPS:
                        return
                    qq = phase_tiles(kk)
                    fscn = fsc_pool.tile([128, W2], F32, tag="fsc",
                                         name=f"fsc{kk}")
                    rounds(kk, qq, fscn)
                    fsc_by[kk] = fscn

                def sc_of(kk):
                    if kk >= NSTEP:
                        return None
                    SCn = sc_pool.tile([128, 2, W2], F16, tag="sc",
                                       name=f"sc{kk}")
                    if kk in BI_STEPS:
                        bi_sc(kk, SCn)
                    else:
                        sincos(kk, fsc_by.pop(kk), SCn)
                    return SCn

                phase_rounds(1)
                phase_rounds(2)
                SC_next = sc_of(1)

                for k in range(1, NSTEP):
                    SC = SC_next
                    t_re, t_im = mm_cast(k, st_re, st_im)
                    SC_next = sc_of(k + 1)
                    st_re, st_im = dmult(k, t_re, t_im, SC)
                    phase_rounds(k + 2)

                # ---- tail: final H, probs, Z-projection
                sq = prod_pool.tile([128, 2 * W2], F16, tag="pA", name="sq")
                probs = st_pool.tile([128, W2], F16, tag="sre",
                                     name="probs")
                for uv in (0, 1):
                    bsl = slice(128, 256) if uv else slice(0, 128)
                    for ci, s_t in ((0, st_re), (1, st_im)):
                        ps = pp.tile([128, H], F32, tag="ps",
                                     name=f"f{ci}{uv}")
                        for c in range(H // 512):
                            sl = slice(c * 512, (c + 1) * 512)
                            nc.tensor.matmul(
                                ps[:, sl], Mt[:, 0:128],
                                s_t[:, c * 512:(c + 1) * 512],
                                start=True, stop=False)
                            nc.tensor.matmul(
                                ps[:, sl], Mt[:, bsl],
                                s_t[:, H + c * 512:H + (c + 1) * 512],
                                start=False, stop=True)
                        nc.scalar.activation(
                            sq[:, ci * W2 + uv * H:ci * W2 + (uv + 1) * H],
                            ps[:], ACTF.Square, scale=1.0 / 16.0)
                    uvs = slice(uv * H, (uv + 1) * H)
                    nc.vector.tensor_add(probs[:, uvs],
                                         sq[:, uv * H:(uv + 1) * H],
                                         sq[:, W2 + uv * H:W2 + (uv + 1) * H])
                zp = pp.tile([128, H], F32, tag="ps", name="zp")
                for c in range(H // 512):
                    csl = slice(c * 512, (c + 1) * 512)
                    nc.tensor.matmul(zp[0:8, csl], Zt[:, 0:8],
                                     probs[:, c * 512:(c + 1) * 512],
                                     start=True, stop=False)
                    nc.tensor.matmul(zp[0:8, csl], Zt[:, 8:16],
                                     probs[:, H + c * 512:H + (c + 1) * 512],
                                     start=False, stop=True)
                zs = fab_pool.tile([8, B_CORE], F32, tag="fab", name="zs")
                nc.scalar.activation(zs[:], zp[0:8, 0:B_CORE], ACTF.Copy)
                nc.sync.dma_start(out_d[:], zs[:])
    _legalize_waits(nc, limit=int(os.environ.get('LW', '1')),
                    limit_other=int(os.environ.get('LWO', '1')))
    return nc


_PROGRAM_CACHE = {}


def make_in_maps(observation, theta, omega):
    observation = np.asarray(observation, np.float32)
    W, CS, M16, Z16 = _build_host_tables(theta, omega)
    x_augT = np.concatenate(
        [observation, np.ones((B_TOTAL, 1), np.float32)], 1).T  # (13, 16384)

    in_maps = []
    for c in range(N_CORES):
        in_maps.append({
            "xT": np.ascontiguousarray(x_augT[:, c * B_CORE:(c + 1) * B_CORE]),
            "W": W,
            "M": M16,
            "Zt": Z16,
            "CS": CS,
        })
    return in_maps


def finalize_output(per_core_out):
    return np.ascontiguousarray(per_core_out.T)


def kernel(observation, theta, omega, _trace=False):
    in_maps = make_in_maps(observation, theta, omega)

    if "nc" not in _PROGRAM_CACHE:
        _PROGRAM_CACHE["nc"] = _build_program()
    nc = _PROGRAM_CACHE["nc"]

    res = run_bass_kernel_spmd(nc, in_maps, core_ids=list(range(N_CORES)),
                               trace=_trace)
    out = np.concatenate([finalize_output(r["out"]) for r in res.results], 0)
    if _trace:
        kernel.last_results = res
    return out


# revision 3
# speedup vs baseline: 1.0404x; 1.0404x over previous
"""Trainium2 Bass kernel for nn_DataReuploadingTorso (8-qubit data-reuploading
circuit, batch 16384), fp16 pipeline v8.

Math: each PennyLane Rot = RZ H RZ H RZ; the circuit becomes 17 diagonal
phase steps interleaved with H^{x8}. Per step k: st' = D_k (*) (H st).

Device mapping (per core, batch 2048; state [128 part = low 7 bits,
(h, batch) free]):
- H-transform: butterfly folded into PSUM accumulation - per 512-chunk
  t_u = M a + M b, t_v = M a + (-M) b with M = H^{x7}/16 fp16 stationary
  ([M | -M] table), fp16 state moving (1 cyc/row).
- PSUM -> SBUF fp16 casts on the ACT engine (Copy), so every DVE
  TensorTensor op is all-16-bit packed -> 2x_1p perf mode.
- Phase pipeline (2-stage lookahead, fully off the state critical path):
  Phi (turns) via fp32r matmul; round n = rne(Phi) as an ACT Copy to
  int16 (hw round-to-nearest); f = Phi - n via one DVE STT; S = Sin(2pi f)
  and C = sin(pi/2 - 2pi |f|) (ACT Abs + Sin with scale=-2pi, bias=pi/2)
  - no second range reduction for cos. Batch-indep steps broadcast host
  cos/sin (sin halves DVE-copy at 4x, cos halves ACT-copy).
- Multiply: pA = t_re (*) [S|C], pB = t_im (*) [S|C] (fp16 2x), then
  per-half sub/add -> new state (fp16). Last stage finishes n_re first
  so the tail's re matmuls start early; tail runs re-component first.
- Emission order per stage: state MMs + casts lead every queue; next
  stage's Sin/Abs/Sin reads an fsc finished one stage earlier; rounds
  for stage k+2 trail (before dmult on BI stages). SC bufs=3 /
  fsc bufs=2 carry the lookahead.

Sharding: pure data-parallel over batch across 8 NeuronCores (2048 each).
"""
import os

import numpy as np

import concourse.bass as bass
import concourse.mybir as mybir
import concourse.tile as tile
from concourse.bass_utils import run_bass_kernel_spmd

N_CORES = 8
B_TOTAL = 16384
B_CORE = B_TOTAL // N_CORES      # 2048
H = B_CORE                       # one half (h) slab of the free dim
W2 = 2 * B_CORE                  # 4096 free cols: [h=0 batch | h=1 batch]
NSTEP = 17
DIM = 256
N_Q = 8

F32 = mybir.dt.float32
F32R = mybir.dt.float32r
F16 = mybir.dt.float16
I16 = mybir.dt.int16
AOT = mybir.AluOpType
ACTF = mybir.ActivationFunctionType

PI = float(np.pi)
TWO_PI = float(np.float32(2.0 * np.pi))
HALF_PI = float(np.pi / 2.0)

KREP = int(os.environ.get("KREP", "1"))
ROUND_MODE = os.environ.get("ROUND_MODE", "act")      # act | dve | split
CAST_IM = os.environ.get("CAST_IM", "act")            # act | skip | pool
PHASE_LATE = os.environ.get("PHASE_LATE", "0") == "1"
COS_VIA = os.environ.get("COS_VIA", "act")            # act (Abs+Sin) | dve (TS)
POOL_BFLY = int(os.environ.get("POOL_BFLY", "0"))     # 0/2/4 bfly units on Pool

BI_STEPS = (2, 6, 10, 14)
BD_STEPS = tuple(k for k in range(NSTEP) if k not in BI_STEPS)  # 13 steps
BD_POS = {k: i for i, k in enumerate(BD_STEPS)}


# ----------------------------------------------------------------- host tables
def _build_host_tables(theta, omega):
    """W (13, 13*256) phase weights (turns) for the 13 batch-dep steps;
    CS (128, 16) cos/sin for the 4 batch-indep steps; M16 (128,128) fp16 =
    H^{x7}/16; Z16 (128, 16) fp16 = [Z_h0 | Z_h1] PauliZ stationaries."""
    theta = np.asarray(theta, np.float64)              # (8, 5, 3)
    omega = np.asarray(omega, np.float64).reshape(5, 8, 3)

    idx = np.arange(DIM)
    beta = np.stack([(idx >> (7 - q)) & 1 for q in range(N_Q)], 0)   # (8, 256)
    sgn = (2 * beta - 1).astype(np.float64)

    def czterm(pairs):
        t = np.zeros(DIM)
        for a, b in pairs:
            t += np.pi * (beta[a] * beta[b])
        return t
    cz_even = czterm([(0, 1), (2, 3), (4, 5), (6, 7)])
    cz_odd = czterm([(1, 2), (3, 4), (5, 6)])

    steps = []
    for l in range(4):
        A = np.zeros((12, 8)); k = np.zeros(8)
        for q in range(8):
            A[3 * (q % 4) + 1, q] = omega[l, q, 1]
        steps.append((A, k, None))
        A = np.zeros((12, 8)); k = np.zeros(8)
        for q in range(8):
            A[3 * (q % 4) + 2, q] = omega[l, q, 2]
            k[q] = theta[q, l, 0]
        steps.append((A, k, None))
        A = np.zeros((12, 8)); k = theta[:, l, 1].copy()
        steps.append((A, k, None))                     # batch-independent
        A = np.zeros((12, 8)); k = theta[:, l, 2].copy()
        for q in range(8):
            A[3 * (q % 4) + 0, q] = omega[l + 1, q, 0]
        if l + 1 == 4:
            k = k + theta[:, 4, 0]
        steps.append((A, k, cz_even if l % 2 == 0 else cz_odd))
    A = np.zeros((12, 8)); k = theta[:, 4, 1].copy()
    for q in range(8):
        A[3 * (q % 4) + 1, q] = omega[4, q, 1]
    steps.append((A, k, None))

    inv2pi = 1.0 / (2.0 * np.pi)
    W = np.zeros((13, len(BD_STEPS) * DIM))
    CS = np.zeros((128, 4 * len(BI_STEPS)), np.float32)
    for i, (A, k, cz) in enumerate(steps):
        c = k @ (sgn * 0.5)
        if cz is not None:
            c = c + cz
        if i in BI_STEPS:
            j = BI_STEPS.index(i)
            for h in (0, 1):
                CS[:, 4 * j + 0 + h] = np.cos(c[h * 128:(h + 1) * 128])
                CS[:, 4 * j + 2 + h] = np.sin(c[h * 128:(h + 1) * 128])
        else:
            base = BD_POS[i] * DIM
            W[:12, base:base + DIM] = (A @ (sgn * 0.5)) * inv2pi
            W[12, base:base + DIM] = \
                (np.mod(c + np.pi, 2 * np.pi) - np.pi) * inv2pi

    sp = np.arange(128)
    pop = np.zeros((128, 128), np.int64)
    for q in range(7):
        pop += np.outer((sp >> q) & 1, (sp >> q) & 1)
    M1 = (np.where(pop % 2 == 0, 1.0, -1.0) / 16.0)
    M16 = np.concatenate([M1, -M1], 1).astype(np.float16)   # [M | -M]

    Z = np.stack([1.0 - 2.0 * ((idx >> (7 - q)) & 1) for q in range(8)], 1)
    Z16 = np.concatenate([Z[:128], Z[128:]], 1).astype(np.float16)  # (128,16)
    return W.astype(np.float32), CS.astype(np.float16), M16, Z16


# -------------------------------------------------------------- device program
def _legalize_waits(nc, limit=1, limit_other=None):
    """walrus codegen allows only one embedded sync-wait on several TRN2
    instruction encodings. Hoist excess waits onto same-engine NoOps."""
    if limit_other is None:
        limit_other = limit
    one_wait = (mybir.InstMatmult, mybir.InstTensorScalarPtr)

    def fix_block(blk):
        new_insts = []
        for ins in blk.instructions:
            lim = limit if isinstance(ins, one_wait) else limit_other
            si = getattr(ins, "sync_info", None)
            waits = list(si.on_wait) if si and si.on_wait else []
            if len(waits) > lim:
                keep = waits[-lim:]
                for j, w in enumerate(waits[:-lim]):
                    new_insts.append(mybir.InstNoOp(
                        name=f"{ins.name}-w{j}",
                        engine=ins.engine,
                        sync_info=mybir.SyncInfo(on_wait=[w], on_update=[]),
                    ))
                si.on_wait = keep
            new_insts.append(ins)
        blk.instructions = new_insts
        for sb in getattr(blk, "blocks", None) or []:
            fix_block(sb)
    for f in nc.m.functions:
        for blk in f.blocks:
            fix_block(blk)


def _build_program():
    nc = bass.Bass("TRN2", target_bir_lowering=False, debug=False,
                   enable_asserts=False, num_devices=N_CORES)

    # const APs for ACT biases (bias floats must be pre-registered)
    for val in (HALF_PI,):
        t = nc.alloc_sbuf_tensor(f"const-{val:.6f}", [128, 1], F32)
        nc.gpsimd.memset(t.ap(), val)
        nc.const_aps.aps[(F32, val)] = t.ap()

    xT_d = nc.dram_tensor("xT", [13, B_CORE], F32, kind="ExternalInput")
    W_d = nc.dram_tensor("W", [13, len(BD_STEPS) * DIM], F32,
                         kind="ExternalInput")
    M_d = nc.dram_tensor("M", [128, 256], F16, kind="ExternalInput")
    Z_d = nc.dram_tensor("Zt", [128, 16], F16, kind="ExternalInput")
    CS_d = nc.dram_tensor("CS", [128, 4 * len(BI_STEPS)], F16,
                          kind="ExternalInput")
    out_d = nc.dram_tensor("out", [8, B_CORE], F32, kind="ExternalOutput")

    with tile.TileContext(nc) as tc:
        with (
            tc.tile_pool(name="consts", bufs=1) as consts,
            tc.tile_pool(name="psum", bufs=2, space="PSUM") as pp,
            tc.tile_pool(name="st", bufs=2) as st_pool,
            tc.tile_pool(name="t16", bufs=1) as t_pool,
            tc.tile_pool(name="sc", bufs=3) as sc_pool,
            tc.tile_pool(name="fsc", bufs=2) as fsc_pool,
            tc.tile_pool(name="fab", bufs=1) as fab_pool,
            tc.tile_pool(name="iq", bufs=2) as iq_pool,
            tc.tile_pool(name="prod", bufs=1) as prod_pool,
        ):
            xT = consts.tile([13, B_CORE], F32R, tag="xT")
            nc.sync.dma_start(xT[:], xT_d[:].bitcast(F32R))
            Wt = consts.tile([13, len(BD_STEPS) * DIM], F32R, tag="W")
            nc.sync.dma_start(Wt[:], W_d[:].bitcast(F32R))
            Mt = consts.tile([128, 256], F16, tag="M")
            nc.sync.dma_start(Mt[:], M_d[:])
            Zt = consts.tile([128, 16], F16, tag="Z")
            nc.sync.dma_start(Zt[:], Z_d[:])
            cst = consts.tile([128, 4 * len(BI_STEPS)], F16, tag="cs")
            nc.sync.dma_start(cst[:], CS_d[:])

            def phase_tiles(k):
                """Phase matmuls for BD step k: two [128, H] psum tiles
                (turns), one per h."""
                base = BD_POS[k] * DIM
                tiles = []
                for h in (0, 1):
                    qp = pp.tile([128, H], F32, tag="ps", name=f"q{k}h{h}")
                    for c in range(H // 512):
                        nc.tensor.matmul(
                            qp[:, c * 512:(c + 1) * 512],
                            Wt[:, base + h * 128: base + (h + 1) * 128],
                            xT[:, c * 512:(c + 1) * 512],
                            start=True, stop=True)
                    tiles.append(qp)
                return tiles

            def rounds(k, qtiles, fsc):
                """fsc[:, h*H:(h+1)*H] = Phi_h - rne(Phi_h)  (turns)."""
                if ROUND_MODE == "split":
                    qi1 = iq_pool.tile([128, H], I16, tag="iq",
                                       name=f"qi{k}h1")
                    nc.vector.tensor_scalar_add(qi1[:], qtiles[1][:], 0.0)
                    qi0 = iq_pool.tile([128, H], I16, tag="iq",
                                       name=f"qi{k}h0")
                    nc.scalar.activation(qi0[:], qtiles[0][:], ACTF.Copy)
                    nc.vector.scalar_tensor_tensor(
                        fsc[:, H:W2], qtiles[1][:], 0.0,
                        qi1[:], AOT.add, AOT.subtract)
                    nc.vector.scalar_tensor_tensor(
                        fsc[:, 0:H], qtiles[0][:], 0.0,
                        qi0[:], AOT.add, AOT.subtract)
                    return
                for h in (0, 1):
                    qi = iq_pool.tile([128, H], I16, tag="iq",
                                      name=f"qi{k}h{h}")
                    if ROUND_MODE == "act":
                        nc.scalar.activation(qi[:], qtiles[h][:], ACTF.Copy)
                    else:
                        nc.vector.tensor_scalar_add(qi[:], qtiles[h][:], 0.0)
                    nc.vector.scalar_tensor_tensor(
                        fsc[:, h * H:(h + 1) * H], qtiles[h][:], 0.0,
                        qi[:], AOT.add, AOT.subtract)

            def sincos(k, fsc, SC):
                """SC[:,0]=sin(2pi f), SC[:,1]=cos fp16; cos via
                sin(pi/2 - 2pi |f|). Per-half so Abs h0 starts right
                after STT h0."""
                for h in (0, 1):
                    hs = slice(h * H, (h + 1) * H)
                    nc.scalar.activation(SC[:, 0, hs], fsc[:, hs], ACTF.Sin,
                                         scale=TWO_PI)
                    fab = fab_pool.tile([128, H], F32, tag="fab",
                                        name=f"fab{k}h{h}")
                    nc.scalar.activation(fab[:], fsc[:, hs], ACTF.Abs)
                    nc.scalar.activation(SC[:, 1, hs], fab[:], ACTF.Sin,
                                         scale=-TWO_PI, bias=HALF_PI)

            def bi_sc(k, SC):
                """Broadcast host cos/sin columns for batch-indep step k
                (sin halves on DVE, cos halves on ACT to split the load)."""
                j = BI_STEPS.index(k)
                for h in (0, 1):
                    nc.vector.tensor_copy(
                        SC[:, 0, h * H:(h + 1) * H],
                        cst[:, 4 * j + 2 + h:4 * j + 3 + h]
                        .to_broadcast((128, H)))
                    nc.scalar.activation(
                        SC[:, 1, h * H:(h + 1) * H],
                        cst[:, 4 * j + h:4 * j + h + 1]
                        .to_broadcast((128, H)), ACTF.Copy)

            def mm_cast(k, s_re, s_im):
                """Butterfly folded into PSUM accumulation: per chunk
                t_u = M a + M b (start/stop pair), t_v = M a + (-M) b.
                Reads the state tiles directly (no DVE butterfly). The re
                component is ACT-cast to fp16; the im component is cast,
                pool-cast, or left in PSUM for 1x muls per CAST_IM."""
                tr = t_pool.tile([128, W2], F16, tag="tr", name=f"tr{k}")
                ti = t_pool.tile([128, W2], F16, tag="ti", name=f"ti{k}")
                im_ps = []
                for uv in (0, 1):
                    bsl = slice(128, 256) if uv else slice(0, 128)
                    for comp, s_t, t16 in (("r", s_re, tr), ("i", s_im, ti)):
                        ps = pp.tile([128, H], F32, tag="ps",
                                     name=f"s{k}{comp}{uv}")
                        for c in range(H // 512):
                            sl = slice(c * 512, (c + 1) * 512)
                            nc.tensor.matmul(
                                ps[:, sl], Mt[:, 0:128],
                                s_t[:, c * 512:(c + 1) * 512],
                                start=True, stop=False)
                            nc.tensor.matmul(
                                ps[:, sl], Mt[:, bsl],
                                s_t[:, H + c * 512:H + (c + 1) * 512],
                                start=False, stop=True)
                        if comp == "i" and CAST_IM == "skip":
                            im_ps.append(ps)
                        elif comp == "i" and CAST_IM == "pool":
                            nc.gpsimd.tensor_copy(
                                ti[:, uv * H:(uv + 1) * H], ps[:])
                        else:
                            nc.scalar.activation(
                                t16[:, uv * H:(uv + 1) * H], ps[:],
                                ACTF.Copy)
                return tr, (im_ps if CAST_IM == "skip" else ti)

            def dmult(k, t_re, t_im, SC, re_first=False):
                """st' = (C + iS) (*) t, all fp16 2x. Per-half ops so the
                h=0 chain (mul+addsub) runs while h=1 is still in matmul/
                cast. SC/pA/pB are [128, 2, W2]: plane 0 = S, plane 1 = C."""
                pA = prod_pool.tile([128, 2, W2], F16, tag="pA",
                                    name=f"pA{k}")
                pB = prod_pool.tile([128, 2, W2], F16, tag="pB",
                                    name=f"pB{k}")
                n_re = st_pool.tile([128, W2], F16, tag="sre",
                                    name=f"sre{k}")
                n_im = st_pool.tile([128, W2], F16, tag="sim",
                                    name=f"sim{k}")
                if re_first:
                    # last stage: finish n_re early so the tail's re
                    # matmuls can start while n_im is still combining
                    for h in (0, 1):
                        hs = slice(h * H, (h + 1) * H)
                        nc.vector.tensor_mul(
                            pA[:, :, hs],
                            t_re[:, hs].unsqueeze(1)
                            .to_broadcast((128, 2, H)),
                            SC[:, :, hs])
                        nc.vector.tensor_mul(
                            pB[:, :, hs],
                            t_im[:, hs].unsqueeze(1)
                            .to_broadcast((128, 2, H)),
                            SC[:, :, hs])
                        nc.vector.tensor_sub(n_re[:, hs], pA[:, 1, hs],
                                             pB[:, 0, hs])
                    for h in (0, 1):
                        hs = slice(h * H, (h + 1) * H)
                        nc.vector.tensor_add(n_im[:, hs], pA[:, 0, hs],
                                             pB[:, 1, hs])
                    return n_re, n_im
                for h in (0, 1):
                    hs = slice(h * H, (h + 1) * H)
                    nc.vector.tensor_mul(
                        pA[:, :, hs],
                        t_re[:, hs].unsqueeze(1).to_broadcast((128, 2, H)),
                        SC[:, :, hs])
                    tiv = (t_im[h][:] if isinstance(t_im, list)
                           else t_im[:, hs])
                    nc.vector.tensor_mul(
                        pB[:, :, hs],
                        tiv.unsqueeze(1).to_broadcast((128, 2, H)),
                        SC[:, :, hs])
                    nc.vector.tensor_sub(n_re[:, hs], pA[:, 1, hs],
                                         pB[:, 0, hs])
                    nc.vector.tensor_add(n_im[:, hs], pA[:, 0, hs],
                                         pB[:, 1, hs])
                return n_re, n_im

            for _rep in range(KREP):
                # ---- init: st = e^{i Phi_0} (x16 folded into M + Square)
                q0 = phase_tiles(0)
                fsc0 = fsc_pool.tile([128, W2], F32, tag="fsc", name="fsc0")
                rounds(0, q0, fsc0)
                st_im = st_pool.tile([128, W2], F16, tag="sim", name="sim0")
                nc.scalar.activation(st_im[:], fsc0[:], ACTF.Sin,
                                     scale=TWO_PI)
                st_re = st_pool.tile([128, W2], F16, tag="sre", name="sre0")
                fab0 = fab_pool.tile([128, W2], F32, tag="fab", name="fab0")
                nc.scalar.activation(fab0[:], fsc0[:], ACTF.Abs)
                nc.scalar.activation(st_re[:], fab0[:], ACTF.Sin,
                                     scale=-TWO_PI, bias=HALF_PI)

                # 2-stage phase lookahead: during stage k we emit
                # phase-matmuls+rounds for k+2 and the Sin/Abs/Sin for k+1
                # (whose fsc was finished last stage) - so ACT work never
                # waits on this stage's DVE.
                fsc_by = {}

                def phase_rounds(kk):
                    if kk >= NSTEP or kk in BI_STEPS:
                        return
                    qq = phase_tiles(kk)
                    fscn = fsc_pool.tile([128, W2], F32, tag="fsc",
                                         name=f"fsc{kk}")
                    rounds(kk, qq, fscn)
                    fsc_by[kk] = fscn

                def sc_of(kk):
                    if kk >= NSTEP:
                        return None
                    SCn = sc_pool.tile([128, 2, W2], F16, tag="sc",
                                       name=f"sc{kk}")
                    if kk in BI_STEPS:
                        bi_sc(kk, SCn)
                    else:
                        sincos(kk, fsc_by.pop(kk), SCn)
                    return SCn

                phase_rounds(1)
                phase_rounds(2)
                SC_next = sc_of(1)

                for k in range(1, NSTEP):
                    SC = SC_next
                    t_re, t_im = mm_cast(k, st_re, st_im)
                    SC_next = sc_of(k + 1)
                    if k in BI_STEPS:
                        # BI stages have a light ACT queue: emit the k+2
                        # round pipeline before dmult so the post-BI Sin
                        # is not starved
                        phase_rounds(k + 2)
                        st_re, st_im = dmult(k, t_re, t_im, SC)
                    else:
                        st_re, st_im = dmult(k, t_re, t_im, SC,
                                             re_first=(k == NSTEP - 1))
                        phase_rounds(k + 2)

                # ---- tail: final H, probs, Z-projection
                sq = prod_pool.tile([128, 2 * W2], F16, tag="pA", name="sq")
                probs = st_pool.tile([128, W2], F16, tag="sre",
                                     name="probs")
                for ci, s_t in ((0, st_re), (1, st_im)):
                    for uv in (0, 1):
                        bsl = slice(128, 256) if uv else slice(0, 128)
                        ps = pp.tile([128, H], F32, tag="ps",
                                     name=f"f{ci}{uv}")
                        for c in range(H // 512):
                            sl = slice(c * 512, (c + 1) * 512)
                            nc.tensor.matmul(
                                ps[:, sl], Mt[:, 0:128],
                                s_t[:, c * 512:(c + 1) * 512],
                                start=True, stop=False)
                            nc.tensor.matmul(
                                ps[:, sl], Mt[:, bsl],
                                s_t[:, H + c * 512:H + (c + 1) * 512],
                                start=False, stop=True)
                        nc.scalar.activation(
                            sq[:, ci * W2 + uv * H:ci * W2 + (uv + 1) * H],
                            ps[:], ACTF.Square, scale=1.0 / 16.0)
                for uv in (0, 1):
                    uvs = slice(uv * H, (uv + 1) * H)
                    nc.vector.tensor_add(probs[:, uvs],
                                         sq[:, uv * H:(uv + 1) * H],
                                         sq[:, W2 + uv * H:W2 + (uv + 1) * H])
                zp = pp.tile([128, H], F32, tag="ps", name="zp")
                for c in range(H // 512):
                    csl = slice(c * 512, (c + 1) * 512)
                    nc.tensor.matmul(zp[0:8, csl], Zt[:, 0:8],
                                     probs[:, c * 512:(c + 1) * 512],
                                     start=True, stop=False)
                    nc.tensor.matmul(zp[0:8, csl], Zt[:, 8:16],
                                     probs[:, H + c * 512:H + (c + 1) * 512],
                                     start=False, stop=True)
                zs = fab_pool.tile([8, B_CORE], F32, tag="fab", name="zs")
                nc.scalar.activation(zs[:], zp[0:8, 0:B_CORE], ACTF.Copy)
                nc.sync.dma_start(out_d[:], zs[:])
    _legalize_waits(nc, limit=int(os.environ.get('LW', '1')),
                    limit_other=int(os.environ.get('LWO', '1')))
    return nc


_PROGRAM_CACHE = {}


def make_in_maps(observation, theta, omega):
    observation = np.asarray(observation, np.float32)
    W, CS, M16, Z16 = _build_host_tables(theta, omega)
    x_augT = np.concatenate(
        [observation, np.ones((B_TOTAL, 1), np.float32)], 1).T  # (13, 16384)

    in_maps = []
    for c in range(N_CORES):
        in_maps.append({
            "xT": np.ascontiguousarray(x_augT[:, c * B_CORE:(c + 1) * B_CORE]),
            "W": W,
            "M": M16,
            "Zt": Z16,
            "CS": CS,
        })
    return in_maps


def finalize_output(per_core_out):
    return np.ascontiguousarray(per_core_out.T)


def kernel(observation, theta, omega, _trace=False):
    in_maps = make_in_maps(observation, theta, omega)

    if "nc" not in _PROGRAM_CACHE:
        _PROGRAM_CACHE["nc"] = _build_program()
    nc = _PROGRAM_CACHE["nc"]

    res = run_bass_kernel_spmd(nc, in_maps, core_ids=list(range(N_CORES)),
                               trace=_trace)
    out = np.concatenate([finalize_output(r["out"]) for r in res.results], 0)
    if _trace:
        kernel.last_results = res
    return out
